# revision 3
# baseline (speedup 1.0000x reference)
"""Trainium2 Bass kernel for masked attention (v2: pipelined, packed QK,
ACT+DVE dual-engine exp).

Reference semantics (B=4, S=4096, D=64):
    qs = q / 8
    scores = qs @ k.T + log(mask)[:, None, :]     # mask keys
    w = softmax(scores, axis=-1)
    out = w @ v
    return out * mask[..., None] + qs * (1 - mask)[..., None]

Sharding: 8 cores = (batch b = c//2, query half h = c%2). Each core
computes attention for 2048 queries of one batch with the batch's full
K/V/mask. Queries are independent -> no collectives.

Per-core algorithm (keys-on-partitions layout), 64 granules of
(128 keys x 1024 queries):
    scoresT[k, q] = sum_d K[k,d] Q[q,d]     2 PE matmuls row/col-packed via
        tile_position (0,0)/(64,0): contraction is only d=64, so two
        512-query streams run concurrently in the two PE row halves.
        Requires kT/qT duplicated on partitions 64-127.
    E = exp(scoresT * 0.125)                granules split between ACT
        (exact exp) and DVE (one-op Schraudolph: int16 = round(sc*A + B)
        bit-synthesizes bf16 2^x; DC bias cancels in softmax).
    outT[m, q] = sum_k Vaug[k, m] E[k, q]   PE accumulate over 32 kb.
        Vaug[:, 0:64] = mask*V, Vaug[:, 64] = mask (denominator row).
    Epilogue: PE-transpose outT, divide by denom row, blend with qs
    passthrough for masked queries.

Pipeline: all prep (DMA, K/Q transposes, Vaug) is interleaved into the
main granule loop; emission order per granule is exp(g), QK(g+1), PV(g)
so the in-order PE never stalls the exp engines. GPSIMD handles the
SBUF-only elementwise prep.
"""

import numpy as np

import concourse.bacc as bacc
import concourse.bass as bass
import concourse.tile as tile
import concourse.mybir as mybir
from concourse.bass_utils import run_bass_kernel_spmd
from concourse.masks import make_identity

B, S, D = 4, 4096, 64
NCORES = 8
QSH = (B * S) // NCORES          # 2048 queries per core
NKB = S // 128                   # 32 key blocks
NQT = QSH // 128                 # 16 query tiles
QCH = 1024                       # query-chunk width (free dim of scoresT)
NQC = QSH // QCH                 # 2 query chunks
NGR = NQC * NKB                  # 64 granules

F32 = mybir.dt.float32
I16 = mybir.dt.int16
BF16 = mybir.dt.bfloat16
Exp = mybir.ActivationFunctionType.Exp
MUL = mybir.AluOpType.mult
ADD = mybir.AluOpType.add

# Schraudolph constants: int16 bits = round(sc*SCH_A + SCH_B) ~ bf16(e^(sc/8))
LOG2E = 1.4426950408889634
SCH_A = 0.125 * LOG2E * 128.0
SCH_B = 127.0 * 128.0 - 5.76     # -5.76: center the sawtooth (C=0.045*128)

# granule -> exp engine: DVE every 3rd granule
DVE_MOD, DVE_PHASE = 3, 2


def _emit(tc, nc, q_d, k_d, v_d, mk_d, mq_d, o_d, dve_mod=DVE_MOD,
          dve_phase=DVE_PHASE):
    consts = tc.alloc_tile_pool(name="consts", bufs=1)
    sb = tc.alloc_tile_pool(name="sb", bufs=1)
    expp = tc.alloc_tile_pool(name="expp", bufs=6)
    finp = tc.alloc_tile_pool(name="finp", bufs=4)
    ps = tc.alloc_tile_pool(name="ps", bufs=3, space="PSUM")     # tag "sc"
    ps_o = tc.alloc_tile_pool(name="ps_o", bufs=1, space="PSUM")  # tag "ot"
    pools = [consts, sb, expp, finp, ps, ps_o]

    identity = consts.tile([128, 128], F32, name="identity")
    make_identity(nc, identity)
    # warm the ACT exp table before the pipeline needs it
    actwarm = consts.tile([1, 1], F32, name="actwarm")
    nc.scalar.activation(out=actwarm, in_=identity[0:1, 0:1], func=Exp)

    q3 = sb.tile([128, NQT, D], F32, name="q3")      # q natural, qt = rows qt*128+p
    k3 = sb.tile([128, NKB, D], F32, name="k3")
    v3f = sb.tile([128, NKB, D], F32, name="v3f")    # fp32 staging for V
    v3 = sb.tile([128, NKB, D + 1], BF16, name="v3")  # V augmented with mask col
    mk = sb.tile([128, NKB], F32, name="mk_sb")
    mq = sb.tile([128, NQT], F32, name="mq_sb")
    s1 = sb.tile([128, NQT], F32, name="s1_sb")      # 0.125*(1-mq)
    qb3 = sb.tile([128, NQT, D], F32, name="qb3")    # qs*(1-mq) passthrough term
    qT2 = sb.tile([128, QSH], BF16, name="qT2")      # rows 64-127 duplicate 0-63
    kT2 = sb.tile([128, S], BF16, name="kT2")
    oT_sb = sb.tile([D + 1, QCH], F32, name="oT_sb")

    # ---- input DMAs: few, big, first-needed first ----
    qap = q_d.ap().rearrange("(p n) d -> p n d", p=128)
    kap = k_d.ap().rearrange("(p n) d -> p n d", p=128)
    vap = v_d.ap().rearrange("(p n) d -> p n d", p=128)
    nc.sync.dma_start(out=k3[:, 0:8, :], in_=kap[:, 0:8, :])
    nc.sync.dma_start(out=q3[:, 0:8, :], in_=qap[:, 0:8, :])
    nc.sync.dma_start(out=mk, in_=mk_d.ap().rearrange("(p n) -> p n", p=128))
    nc.sync.dma_start(out=mq, in_=mq_d.ap().rearrange("(p n) -> p n", p=128))
    nc.sync.dma_start(out=v3f[:, 0:16, :], in_=vap[:, 0:16, :])
    nc.sync.dma_start(out=k3[:, 8:32, :], in_=kap[:, 8:32, :])
    nc.sync.dma_start(out=q3[:, 8:16, :], in_=qap[:, 8:16, :])
    nc.sync.dma_start(out=v3f[:, 16:32, :], in_=vap[:, 16:32, :])

    # s1 = 0.125 * (1 - mq) = mq * (-0.125) + 0.125   (gpsimd: SBUF-only)
    nc.gpsimd.tensor_scalar(s1, mq, -0.125, 0.125, MUL, ADD)

    def v3_prep(h):
        # Vaug for kb 8h..8h+7: col D = mask, cols 0:D = mask * V
        nc.gpsimd.tensor_copy(v3[:, 8 * h:8 * h + 8, D:D + 1],
                              mk[:, 8 * h:8 * h + 8].rearrange(
                                  "p (n o) -> p n o", o=1))
        for j in range(8):
            kb = 8 * h + j
            nc.gpsimd.tensor_scalar_mul(v3[:, kb, 0:D], v3f[:, kb, :],
                                        mk[:, kb:kb + 1])

    def qb3_prep(quarter):
        for j in range(4):
            qt = 4 * quarter + j
            nc.gpsimd.tensor_scalar_mul(qb3[:, qt, :], q3[:, qt, :],
                                        s1[:, qt:qt + 1])

    def prep_kt(gi):
        # transpose kb 8gi..8gi+7 into kT2[0:64], DMA-duplicate into [64:128]
        tp = ps.tile([64, 1024], F32, name=f"prepk{gi}", tag="sc")
        for j in range(8):
            kb = 8 * gi + j
            nc.tensor.transpose(tp[:, 128 * j:128 * (j + 1)], k3[:, kb, :],
                                identity)
        lo = kT2[0:64, 1024 * gi:1024 * (gi + 1)]
        nc.vector.tensor_copy(lo, tp)
        nc.sync.dma_start(out=kT2[64:128, 1024 * gi:1024 * (gi + 1)], in_=lo)

    def prep_qt(gi):
        tp = ps.tile([64, 1024], F32, name=f"prepq{gi}", tag="sc")
        for j in range(8):
            qt = 8 * gi + j
            nc.tensor.transpose(tp[:, 128 * j:128 * (j + 1)], q3[:, qt, :],
                                identity)
        lo = qT2[0:64, 1024 * gi:1024 * (gi + 1)]
        nc.vector.tensor_copy(lo, tp)
        nc.sync.dma_start(out=qT2[64:128, 1024 * gi:1024 * (gi + 1)], in_=lo)

    # minimal pre-loop prep: first K group, first Q chunk, first Vaug group
    prep_kt(0)
    prep_qt(0)
    v3_prep(0)

    oT = [None, None]

    def emit_qk(g):
        qc, kb = g // NKB, g % NKB
        sc = ps.tile([128, QCH], F32, name=f"sc{g}", tag="sc")
        nc.tensor.matmul(
            sc[:, 0:512],
            lhsT=kT2[0:64, 128 * kb:128 * (kb + 1)],
            rhs=qT2[0:64, QCH * qc:QCH * qc + 512],
            start=True, stop=True, tile_position=(0, 0))
        nc.tensor.matmul(
            sc[:, 512:1024],
            lhsT=kT2[64:128, 128 * kb:128 * (kb + 1)],
            rhs=qT2[64:128, QCH * qc + 512:QCH * qc + 1024],
            start=True, stop=True, tile_position=(64, 0))
        return sc

    def emit_exp(g, sc):
        if g % dve_mod == dve_phase:
            exi = expp.tile([128, QCH], I16, name=f"ex{g}", tag="ex")
            nc.vector.tensor_scalar(exi, sc, SCH_A, SCH_B, MUL, ADD)
            return exi.bitcast(BF16)
        ex = expp.tile([128, QCH], BF16, name=f"ex{g}", tag="ex")
        nc.scalar.activation(out=ex, in_=sc, func=Exp, scale=0.125)
        return ex

    def emit_pv(g, ex):
        qc, kb = g // NKB, g % NKB
        if kb == 0:
            oT[qc] = ps_o.tile([D + 1, QCH], F32, name=f"oT{qc}", tag="ot")
        for j in range(2):
            nc.tensor.matmul(
                oT[qc][:, 512 * j:512 * (j + 1)],
                lhsT=v3[:, kb, :],
                rhs=ex[:, 512 * j:512 * (j + 1)],
                start=(kb == 0), stop=(kb == NKB - 1))

    def emit_epilogue(qc):
        # oT -> SBUF, per-qt transpose back, divide by denom, blend, DMA out
        nc.vector.tensor_copy(oT_sb, oT[qc])
        epi = ps.tile([128, QCH // 128, 128], F32, name=f"epi{qc}", tag="sc")
        fin3 = finp.tile([128, QCH // 128, D], F32, name=f"fin3_{qc}",
                         tag="fin3", bufs=2)
        for t in range(QCH // 128):
            qt = qc * (QCH // 128) + t
            nc.tensor.transpose(epi[:, t, 0:D + 1],
                                oT_sb[:, 128 * t:128 * (t + 1)],
                                identity[0:D + 1, 0:D + 1])
            rec = finp.tile([128, 1], F32, name=f"rec{qt}", tag="rec")
            nc.vector.reciprocal(rec, epi[:, t, D:D + 1])
            recm = finp.tile([128, 1], F32, name=f"recm{qt}", tag="recm")
            nc.vector.tensor_scalar_mul(recm, rec, mq[:, qt:qt + 1])
            # fin = (pv * recm) + qb
            nc.vector.scalar_tensor_tensor(fin3[:, t, :], epi[:, t, 0:D], recm,
                                           qb3[:, qt, :], MUL, ADD)
        oap = o_d.ap().rearrange("(p n) d -> p n d", p=128)
        nc.sync.dma_start(
            out=oap[:, qc * (QCH // 128):(qc + 1) * (QCH // 128), :], in_=fin3)

    # interleaved prep: granule index -> thunk
    inter = {
        1: lambda: prep_kt(1),
        3: lambda: v3_prep(1),
        5: lambda: prep_kt(2),
        7: lambda: v3_prep(2),
        9: lambda: prep_kt(3),
        11: lambda: v3_prep(3),
        13: lambda: prep_qt(1),
        24: lambda: qb3_prep(0),
        26: lambda: qb3_prep(1),
        28: lambda: qb3_prep(2),
        30: lambda: qb3_prep(3),
        34: lambda: emit_epilogue(0),
    }

    # ---- main loop: exp(g) / QK(g+1) / PV(g) ----
    sc_cur = emit_qk(0)
    for g in range(NGR):
        ex = emit_exp(g, sc_cur)
        if g + 1 < NGR:
            sc_cur = emit_qk(g + 1)
        emit_pv(g, ex)
        thunk = inter.get(g)
        if thunk is not None:
            thunk()
    emit_epilogue(1)

    for p in reversed(pools):
        p.release()


_PROGS = {}


def _build(repeat=1, loop=None, **emit_kwargs):
    key = (repeat, loop, tuple(sorted(emit_kwargs.items())))
    if key in _PROGS:
        return _PROGS[key]
    nc = bacc.Bacc("TRN2", target_bir_lowering=False, debug=False)
    q_d = nc.dram_tensor("q_in", [QSH, D], F32, kind="ExternalInput")
    k_d = nc.dram_tensor("k_in", [S, D], F32, kind="ExternalInput")
    v_d = nc.dram_tensor("v_in", [S, D], F32, kind="ExternalInput")
    mk_d = nc.dram_tensor("mk_in", [S], F32, kind="ExternalInput")
    mq_d = nc.dram_tensor("mq_in", [QSH], F32, kind="ExternalInput")
    o_d = nc.dram_tensor("o_out", [QSH, D], F32, kind="ExternalOutput")
    with tile.TileContext(nc) as tc:
        if loop is not None:
            with tc.For_i(0, loop, 1):
                for _ in range(repeat):
                    _emit(tc, nc, q_d, k_d, v_d, mk_d, mq_d, o_d, **emit_kwargs)
        else:
            for _ in range(repeat):
                _emit(tc, nc, q_d, k_d, v_d, mk_d, mq_d, o_d, **emit_kwargs)
    nc.compile()
    _PROGS[key] = nc
    return nc


def make_in_maps(q, k, v, mask):
    q = np.ascontiguousarray(np.asarray(q, dtype=np.float32))
    k = np.ascontiguousarray(np.asarray(k, dtype=np.float32))
    v = np.ascontiguousarray(np.asarray(v, dtype=np.float32))
    mask = np.ascontiguousarray(np.asarray(mask, dtype=np.float32))
    in_maps = []
    for c in range(NCORES):
        b, h = c // 2, c % 2
        sl = slice(h * QSH, (h + 1) * QSH)
        in_maps.append({
            "q_in": np.ascontiguousarray(q[b, sl, :]),
            "k_in": np.ascontiguousarray(k[b]),
            "v_in": np.ascontiguousarray(v[b]),
            "mk_in": np.ascontiguousarray(mask[b]),
            "mq_in": np.ascontiguousarray(mask[b, sl]),
        })
    return in_maps


def gather(results):
    out = np.empty((B, S, D), np.float32)
    for c in range(NCORES):
        b, h = c // 2, c % 2
        out[b, h * QSH:(h + 1) * QSH, :] = results[c]["o_out"]
    return out


def kernel(q, k, v, mask, _spmd_kwargs=None):
    nc = _build()
    in_maps = make_in_maps(q, k, v, mask)
    res = run_bass_kernel_spmd(nc, in_maps, core_ids=list(range(NCORES)),
                               **(_spmd_kwargs or {}))
    out = gather(res.results)
    if _spmd_kwargs:
        kernel._last_results = res
    return out


# revision 26
# speedup vs baseline: 1.3117x; 1.3117x over previous
"""Trainium2 Bass kernel for masked attention (v2: pipelined, packed QK,
ACT+DVE dual-engine exp).

Reference semantics (B=4, S=4096, D=64):
    qs = q / 8
    scores = qs @ k.T + log(mask)[:, None, :]     # mask keys
    w = softmax(scores, axis=-1)
    out = w @ v
    return out * mask[..., None] + qs * (1 - mask)[..., None]

Sharding: 8 cores = (batch b = c//2, query half h = c%2). Each core
computes attention for 2048 queries of one batch with the batch's full
K/V/mask. Queries are independent -> no collectives.

Per-core algorithm (keys-on-partitions layout), 64 granules of
(128 keys x 1024 queries):
    scoresT[k, q] = sum_d K[k,d] Q[q,d]     2 PE matmuls row/col-packed via
        tile_position (0,0)/(64,0): contraction is only d=64, so two
        512-query streams run concurrently in the two PE row halves.
        Requires kT/qT duplicated on partitions 64-127.
    E = exp(scoresT * 0.125)                granules split between ACT
        (exact exp) and DVE (one-op Schraudolph: int16 = round(sc*A + B)
        bit-synthesizes bf16 2^x; DC bias cancels in softmax).
    outT[m, q] = sum_k Vaug[k, m] E[k, q]   PE accumulate over 32 kb.
        Vaug[:, 0:64] = mask*V, Vaug[:, 64] = mask (denominator row).
    Epilogue: PE-transpose outT, divide by denom row, blend with qs
    passthrough for masked queries.

Pipeline: all prep (DMA, K/Q transposes, Vaug) is interleaved into the
main granule loop; emission order per granule is exp(g), QK(g+1), PV(g)
so the in-order PE never stalls the exp engines. GPSIMD handles the
SBUF-only elementwise prep.
"""

import numpy as np

import concourse.bacc as bacc
import concourse.bass as bass
import concourse.tile as tile
import concourse.mybir as mybir
from concourse.bass_utils import run_bass_kernel_spmd
from concourse.masks import make_identity

B, S, D = 4, 4096, 64
NCORES = 8
QSH = (B * S) // NCORES          # 2048 queries per core
NKB = S // 128                   # 32 key blocks
NQT = QSH // 128                 # 16 query tiles
QCH = 1024                       # query-chunk width (free dim of scoresT)
NQC = QSH // QCH                 # 2 query chunks
NGR = NQC * NKB                  # 64 granules

F32 = mybir.dt.float32
I16 = mybir.dt.int16
BF16 = mybir.dt.bfloat16
Exp = mybir.ActivationFunctionType.Exp
MUL = mybir.AluOpType.mult
ADD = mybir.AluOpType.add

# Schraudolph constants: int16 bits = round(sc*SCH_A + SCH_B) ~ bf16(e^(sc/8))
LOG2E = 1.4426950408889634
SCH_A = 0.125 * LOG2E * 128.0
SCH_B = 127.0 * 128.0 - 5.76 - 0.5   # sawtooth centering + HW rounding bias (tuned)

# granule -> exp engine: DVE every 2nd granule
DVE_MOD, DVE_PHASE = 2, 1


def _emit(tc, nc, q_d, k_d, v_d, mk_d, mq_d, o_d, dve_mod=DVE_MOD,
          dve_phase=DVE_PHASE, pack=True, use_gpsimd=True, mask_in_exp=True,
          bf16_kq=True, dup=(), half=(), sch_b_off=0.0, pv_delay=True,
          epi_spread=True, dve_res=(1, 5)):
    ew = nc.gpsimd if use_gpsimd else nc.vector
    KQDT = BF16 if bf16_kq else F32
    consts = tc.alloc_tile_pool(name="consts", bufs=1)
    sb = tc.alloc_tile_pool(name="sb", bufs=1)
    expp = tc.alloc_tile_pool(name="expp", bufs=6)
    finp = tc.alloc_tile_pool(name="finp", bufs=4)
    ps = tc.alloc_tile_pool(name="ps", bufs=3, space="PSUM")     # tag "sc"
    ps_o = tc.alloc_tile_pool(name="ps_o", bufs=1, space="PSUM")  # tag "ot"
    pools = [consts, sb, expp, finp, ps, ps_o]

    identity = consts.tile([128, 128], F32, name="identity")
    make_identity(nc, identity)
    # warm the ACT exp table before the pipeline needs it
    actwarm = consts.tile([1, 1], F32, name="actwarm")
    nc.scalar.activation(out=actwarm, in_=identity[0:1, 0:1], func=Exp)

    q3 = sb.tile([128, NQT, D], KQDT, name="q3")     # q natural, qt = rows qt*128+p
    k3 = sb.tile([128, NKB, D], KQDT, name="k3")
    if bf16_kq:
        identb = consts.tile([128, 128], KQDT, name="identb")
        nc.vector.tensor_copy(identb, identity)
    else:
        identb = identity
    v3f = sb.tile([128, NKB, D], F32, name="v3f")    # fp32 staging for V
    v3 = sb.tile([128, NKB, D + 1], BF16, name="v3")  # V augmented with mask col
    mk = sb.tile([128, NKB], F32, name="mk_sb")
    mq = sb.tile([128, NQT], F32, name="mq_sb")
    s1 = sb.tile([128, NQT], F32, name="s1_sb")      # 0.125*(1-mq)
    lm = sb.tile([128, NKB], F32, name="lm_sb")      # 0 keep / -1e30 masked
    bv = sb.tile([128, NKB], F32, name="bv_sb")      # SCH_B keep / 0 masked
    qb3 = sb.tile([128, NQT, D], F32, name="qb3")    # qs*(1-mq) passthrough term
    k3f = sb.tile([128, 2, D], F32, name="k3f")      # ramp: first 2 kb via HWDGE
    q3f = sb.tile([128, 8, D], F32, name="q3f")      # ramp: first q chunk via HWDGE
    qT2 = sb.tile([128, QSH], BF16, name="qT2")      # rows 64-127 duplicate 0-63
    kT2 = sb.tile([128, S], BF16, name="kT2")
    oT_sb = sb.tile([D + 1, QCH], F32, name="oT_sb")

    # ---- input DMAs: few, big, first-needed first ----
    qap = q_d.ap().rearrange("(p n) d -> p n d", p=128)
    kap = k_d.ap().rearrange("(p n) d -> p n d", p=128)
    vap = v_d.ap().rearrange("(p n) d -> p n d", p=128)
    nc.sync.dma_start(out=k3f, in_=kap[:, 0:2, :])
    nc.sync.dma_start(out=q3f, in_=qap[:, 0:8, :])
    kqdma = nc.gpsimd.dma_start if bf16_kq else nc.sync.dma_start
    kqdma(out=k3[:, 0:8, :], in_=kap[:, 0:8, :])
    kqdma(out=q3[:, 0:8, :], in_=qap[:, 0:8, :])
    nc.sync.dma_start(out=mk, in_=mk_d.ap().rearrange("(p n) -> p n", p=128))
    nc.sync.dma_start(out=mq, in_=mq_d.ap().rearrange("(p n) -> p n", p=128))
    nc.sync.dma_start(out=v3f[:, 0:16, :], in_=vap[:, 0:16, :])
    kqdma(out=k3[:, 8:32, :], in_=kap[:, 8:32, :])
    kqdma(out=q3[:, 8:16, :], in_=qap[:, 8:16, :])
    nc.sync.dma_start(out=v3f[:, 16:32, :], in_=vap[:, 16:32, :])

    # s1 = 0.125 * (1 - mq) = mq * (-0.125) + 0.125   (gpsimd: SBUF-only)
    ew.tensor_scalar(s1, mq, -0.125, 0.125, MUL, ADD)
    if mask_in_exp:
        # lm = (mk-1)*1e30 -> {0, -1e30}; bv = mk*SCH_B -> {SCH_B, 0}
        ew.tensor_scalar(lm, mk, 1e30, -1e30, MUL, ADD)
        # masked: bits = A*sc + 4096 stays positive-small -> bf16 ~2^-96
        ew.tensor_scalar(bv, mk, SCH_B + sch_b_off - 4096.0, 4096.0, MUL, ADD)

    def v3_prep(h):
        if mask_in_exp:
            # mask rides in exp: Vaug = plain V (bf16) + ones column
            ew.memset(v3[:, 8 * h:8 * h + 8, D:D + 1], 1.0)
            ew.tensor_copy(v3[:, 8 * h:8 * h + 8, 0:D],
                           v3f[:, 8 * h:8 * h + 8, :])
            return
        # Vaug for kb 8h..8h+7: col D = mask, cols 0:D = mask * V
        ew.tensor_copy(v3[:, 8 * h:8 * h + 8, D:D + 1],
                              mk[:, 8 * h:8 * h + 8].rearrange(
                                  "p (n o) -> p n o", o=1))
        for j in range(8):
            kb = 8 * h + j
            ew.tensor_scalar_mul(v3[:, kb, 0:D], v3f[:, kb, :],
                                        mk[:, kb:kb + 1])

    def qb3_prep(quarter):
        for j in range(4):
            qt = 4 * quarter + j
            ew.tensor_scalar_mul(qb3[:, qt, :], q3[:, qt, :],
                                        s1[:, qt:qt + 1])

    def micro_prep():
        # ramp fast path: kb0-1 + full first q chunk from f32 staging.
        # Transposes must write PSUM partition 0 (walrus rule); the hi
        # halves come from cross-partition DVE copies (verified on HW).
        tpq = ps.tile([128, 1024], F32, name="tpq_micro", tag="sc")
        tpk = ps.tile([128, 256], F32, name="tpk_micro", tag="sc")
        for j in range(8):
            nc.tensor.transpose(tpq[0:64, 128 * j:128 * (j + 1)],
                                q3f[:, j, :], identity)
        for j in range(2):
            nc.tensor.transpose(tpk[0:64, 128 * j:128 * (j + 1)],
                                k3f[:, j, :], identity)
        nc.vector.tensor_copy(qT2[0:64, 0:512], tpq[0:64, 0:512])
        nc.vector.tensor_copy(kT2[0:64, 0:256], tpk[0:64, :])
        if pack:
            nc.vector.tensor_copy(qT2[64:128, 512:1024], tpq[0:64, 512:1024])
            nc.vector.tensor_copy(kT2[64:128, 0:256], tpk[0:64, :])
        else:
            nc.vector.tensor_copy(qT2[0:64, 512:1024], tpq[0:64, 512:1024])

    def prep_kt(gi, kb_lo=0):
        # transpose kb 8gi+kb_lo..8gi+7 into kT2[0:64]; duplicate into
        # [64:128] (group 0: cross-partition DVE copy, no DMA latency; later
        # groups: SBUF->SBUF DMA off the critical path)
        tp = ps.tile([128, 1024], KQDT, name=f"prepk{gi}", tag="sc")
        for j in range(kb_lo, 8):
            kb = 8 * gi + j
            nc.tensor.transpose(tp[0:64, 128 * j:128 * (j + 1)], k3[:, kb, :],
                                identb)
        lo = kT2[0:64, 1024 * gi + 128 * kb_lo:1024 * (gi + 1)]
        nc.vector.tensor_copy(lo, tp[0:64, 128 * kb_lo:])
        if pack:
            hi = kT2[64:128, 1024 * gi + 128 * kb_lo:1024 * (gi + 1)]
            if gi == 0:
                nc.vector.tensor_copy(hi, tp[0:64, 128 * kb_lo:])
            else:
                nc.sync.dma_start(out=hi, in_=lo)

    def prep_qt(gi):
        # QK tile-A only reads qT2[0:64, chunk+0:512]; tile-B only reads
        # qT2[64:128, chunk+512:1024]. Transpose each qt straight into the
        # half that will read it -> half the copy work, no duplication.
        tp = ps.tile([128, 1024], KQDT, name=f"prepq{gi}", tag="sc")
        base = 1024 * gi
        for j in range(8):
            qt = 8 * gi + j
            nc.tensor.transpose(tp[0:64, 128 * j:128 * (j + 1)],
                                q3[:, qt, :], identb)
        nc.vector.tensor_copy(qT2[0:64, base:base + 512], tp[0:64, 0:512])
        if pack:
            nc.vector.tensor_copy(qT2[64:128, base + 512:base + 1024],
                                  tp[0:64, 512:1024])
        else:
            nc.vector.tensor_copy(qT2[0:64, base + 512:base + 1024],
                                  tp[0:64, 512:1024])

    # minimal pre-loop prep: micro (kb0-1 + q chunk 0), then bulk kb2-7
    micro_prep()
    prep_kt(0, kb_lo=2)
    v3_prep(0)

    oT = [None, None]

    def emit_qk(g):
        qc, kb = g // NKB, g % NKB
        sc = ps.tile([128, QCH], F32, name=f"sc{g}", tag="sc")
        nc.tensor.matmul(
            sc[:, 0:512],
            lhsT=kT2[0:64, 128 * kb:128 * (kb + 1)],
            rhs=qT2[0:64, QCH * qc:QCH * qc + 512],
            start=True, stop=True, tile_position=(0, 0))
        hb = 64 if pack else 0
        if "qk" not in half:
            nc.tensor.matmul(
                sc[:, 512:1024],
                lhsT=kT2[hb:hb + 64, 128 * kb:128 * (kb + 1)],
                rhs=qT2[hb:hb + 64, QCH * qc + 512:QCH * qc + 1024],
                start=True, stop=True, tile_position=(hb, 0))
        if "qk" in dup:
            nc.tensor.matmul(
                sc[:, 0:512], lhsT=kT2[0:64, 128 * kb:128 * (kb + 1)],
                rhs=qT2[0:64, QCH * qc:QCH * qc + 512],
                start=True, stop=True, tile_position=(0, 0))
            nc.tensor.matmul(
                sc[:, 512:1024], lhsT=kT2[hb:hb + 64, 128 * kb:128 * (kb + 1)],
                rhs=qT2[hb:hb + 64, QCH * qc + 512:QCH * qc + 1024],
                start=True, stop=True, tile_position=(hb, 0))
        return sc

    if dve_res is not None:
        dve_set = set(g for g in range(NGR) if g % 8 in dve_res)
    else:
        dve_set = set(g for g in range(NGR)
                      if dve_mod and g % dve_mod == dve_phase)
    # keep DVE clear while it chews the qc0 epilogue burst
    if epi_spread:
        for g_excl, g_repl in ((35, 33), (38, 39)):
            if g_excl in dve_set:
                dve_set.discard(g_excl)
                dve_set.add(g_repl)

    def emit_exp(g, sc):
        kb = g % NKB
        if g in dve_set:
            exi = expp.tile([128, QCH], I16, name=f"ex{g}", tag="ex")
            schb = bv[:, kb:kb + 1] if mask_in_exp else SCH_B + sch_b_off
            nc.vector.tensor_scalar(exi[:, 0:512] if "exp" in half else exi,
                                    sc[:, 0:512] if "exp" in half else sc,
                                    SCH_A, schb, MUL, ADD)
            if "exp" in dup:
                nc.vector.tensor_scalar(exi, sc, SCH_A, schb, MUL, ADD)
            return exi.bitcast(BF16)
        ex = expp.tile([128, QCH], BF16, name=f"ex{g}", tag="ex")
        bias = lm[:, kb:kb + 1] if mask_in_exp else 0.0
        nc.scalar.activation(out=ex[:, 0:512] if "exp" in half else ex,
                             in_=sc[:, 0:512] if "exp" in half else sc,
                             func=Exp, scale=0.125, bias=bias)
        if "exp" in dup:
            nc.scalar.activation(out=ex, in_=sc, func=Exp, scale=0.125, bias=bias)
        return ex

    def emit_pv(g, ex):
        qc, kb = g // NKB, g % NKB
        if kb == 0:
            oT[qc] = ps_o.tile([D + 1, QCH], F32, name=f"oT{qc}", tag="ot")
        for _r in range(2 if "pv" in dup else 1):
            for j in range(1 if "pv" in half else 2):
                nc.tensor.matmul(
                    oT[qc][:, 512 * j:512 * (j + 1)],
                    lhsT=v3[:, kb, :],
                    rhs=ex[:, 512 * j:512 * (j + 1)],
                    start=(kb == 0 and _r == 0), stop=(kb == NKB - 1))

    epist = {}

    def emit_epilogue(qc, chunk=None, nchunks=1):
        # oT -> SBUF, per-qt transpose back, divide by denom, blend, DMA out
        nqt = QCH // 128
        if chunk is None or chunk == 0:
            nc.vector.tensor_copy(oT_sb, oT[qc])
            epist[qc] = (
                ps.tile([128, nqt, 128], F32, name=f"epi{qc}", tag="sc"),
                finp.tile([128, nqt, D], F32, name=f"fin3_{qc}", tag="fin3",
                          bufs=2))
        epi, fin3 = epist[qc]
        per = nqt // nchunks
        ts = range(nqt) if chunk is None else range(chunk * per,
                                                   (chunk + 1) * per)
        for t in ts:
            qt = qc * nqt + t
            nc.tensor.transpose(epi[:, t, 0:D + 1],
                                oT_sb[:, 128 * t:128 * (t + 1)],
                                identity[0:D + 1, 0:D + 1])
            rec = finp.tile([128, 1], F32, name=f"rec{qt}", tag="rec")
            nc.vector.reciprocal(rec, epi[:, t, D:D + 1])
            recm = finp.tile([128, 1], F32, name=f"recm{qt}", tag="recm")
            nc.vector.tensor_scalar_mul(recm, rec, mq[:, qt:qt + 1])
            # fin = (pv * recm) + qb
            nc.vector.scalar_tensor_tensor(fin3[:, t, :], epi[:, t, 0:D], recm,
                                           qb3[:, qt, :], MUL, ADD)
        if chunk is None or chunk == nchunks - 1:
            oap = o_d.ap().rearrange("(p n) d -> p n d", p=128)
            nc.sync.dma_start(
                out=oap[:, qc * nqt:(qc + 1) * nqt, :], in_=fin3)

    # interleaved prep: granule index -> thunk
    inter = {
        1: lambda: prep_kt(1),
        3: lambda: v3_prep(1),
        5: lambda: prep_kt(2),
        7: lambda: v3_prep(2),
        9: lambda: prep_kt(3),
        11: lambda: v3_prep(3),
        13: lambda: prep_qt(1),
        24: lambda: qb3_prep(0),
        26: lambda: qb3_prep(1),
        28: lambda: qb3_prep(2),
        30: lambda: qb3_prep(3),
        34: lambda: emit_epilogue(0, 0, 4),
        35: lambda: emit_epilogue(0, 1, 4),
        36: lambda: emit_epilogue(0, 2, 4),
        37: lambda: emit_epilogue(0, 3, 4),
    } if epi_spread else {
        1: lambda: prep_kt(1),
        3: lambda: v3_prep(1),
        5: lambda: prep_kt(2),
        7: lambda: v3_prep(2),
        9: lambda: prep_kt(3),
        11: lambda: v3_prep(3),
        13: lambda: prep_qt(1),
        24: lambda: qb3_prep(0),
        26: lambda: qb3_prep(1),
        28: lambda: qb3_prep(2),
        30: lambda: qb3_prep(3),
        34: lambda: emit_epilogue(0),
    }

    # ---- main loop: exp(g) / QK(g+1) / PV(g-1) ----
    # PV delayed one granule so the in-order PE runs QK(g+1) before
    # PV(g-1); exp(g+1) then never waits behind PV in the PE stream.
    sc_cur = emit_qk(0)
    ex_prev = None
    for g in range(NGR):
        ex = emit_exp(g, sc_cur)
        if g + 1 < NGR:
            sc_cur = emit_qk(g + 1)
        if pv_delay:
            if ex_prev is not None:
                emit_pv(g - 1, ex_prev)
            ex_prev = ex
        else:
            emit_pv(g, ex)
        thunk = inter.get(g)
        if thunk is not None:
            thunk()
    if pv_delay:
        emit_pv(NGR - 1, ex_prev)
    emit_epilogue(1)

    for p in reversed(pools):
        p.release()


_PROGS = {}


def _build(repeat=1, loop=None, **emit_kwargs):
    key = (repeat, loop, tuple(sorted(emit_kwargs.items())))
    if key in _PROGS:
        return _PROGS[key]
    nc = bacc.Bacc("TRN2", target_bir_lowering=False, debug=False)
    q_d = nc.dram_tensor("q_in", [QSH, D], F32, kind="ExternalInput")
    k_d = nc.dram_tensor("k_in", [S, D], F32, kind="ExternalInput")
    v_d = nc.dram_tensor("v_in", [S, D], F32, kind="ExternalInput")
    mk_d = nc.dram_tensor("mk_in", [S], F32, kind="ExternalInput")
    mq_d = nc.dram_tensor("mq_in", [QSH], F32, kind="ExternalInput")
    o_d = nc.dram_tensor("o_out", [QSH, D], F32, kind="ExternalOutput")
    with tile.TileContext(nc) as tc:
        if loop is not None:
            with tc.For_i(0, loop, 1):
                for _ in range(repeat):
                    _emit(tc, nc, q_d, k_d, v_d, mk_d, mq_d, o_d, **emit_kwargs)
        else:
            for _ in range(repeat):
                _emit(tc, nc, q_d, k_d, v_d, mk_d, mq_d, o_d, **emit_kwargs)
    nc.compile()
    _PROGS[key] = nc
    return nc


def make_in_maps(q, k, v, mask):
    q = np.ascontiguousarray(np.asarray(q, dtype=np.float32))
    k = np.ascontiguousarray(np.asarray(k, dtype=np.float32))
    v = np.ascontiguousarray(np.asarray(v, dtype=np.float32))
    mask = np.ascontiguousarray(np.asarray(mask, dtype=np.float32))
    in_maps = []
    for c in range(NCORES):
        b, h = c // 2, c % 2
        sl = slice(h * QSH, (h + 1) * QSH)
        in_maps.append({
            "q_in": np.ascontiguousarray(q[b, sl, :]),
            "k_in": np.ascontiguousarray(k[b]),
            "v_in": np.ascontiguousarray(v[b]),
            "mk_in": np.ascontiguousarray(mask[b]),
            "mq_in": np.ascontiguousarray(mask[b, sl]),
        })
    return in_maps


def gather(results):
    out = np.empty((B, S, D), np.float32)
    for c in range(NCORES):
        b, h = c // 2, c % 2
        out[b, h * QSH:(h + 1) * QSH, :] = results[c]["o_out"]
    return out


def kernel(q, k, v, mask, _spmd_kwargs=None):
    nc = _build()
    in_maps = make_in_maps(q, k, v, mask)
    res = run_bass_kernel_spmd(nc, in_maps, core_ids=list(range(NCORES)),
                               **(_spmd_kwargs or {}))
    out = gather(res.results)
    if _spmd_kwargs:
        kernel._last_results = res
    return out


# revision 29
# speedup vs baseline: 1.3303x; 1.0141x over previous
"""Trainium2 Bass kernel for masked attention (v2: pipelined, packed QK,
ACT+DVE dual-engine exp).

Reference semantics (B=4, S=4096, D=64):
    qs = q / 8
    scores = qs @ k.T + log(mask)[:, None, :]     # mask keys
    w = softmax(scores, axis=-1)
    out = w @ v
    return out * mask[..., None] + qs * (1 - mask)[..., None]

Sharding: 8 cores = (batch b = c//2, query half h = c%2). Each core
computes attention for 2048 queries of one batch with the batch's full
K/V/mask. Queries are independent -> no collectives.

Per-core algorithm (keys-on-partitions layout), 64 granules of
(128 keys x 1024 queries):
    scoresT[k, q] = sum_d K[k,d] Q[q,d]     2 PE matmuls row/col-packed via
        tile_position (0,0)/(64,0): contraction is only d=64, so two
        512-query streams run concurrently in the two PE row halves.
        Requires kT/qT duplicated on partitions 64-127.
    E = exp(scoresT * 0.125)                every granule split by query
        columns: ACT does cols 0:640 (exact exp, per-key log-mask bias),
        DVE does cols 640:1024 (one-op Schraudolph: int16 = round(sc*A+B)
        bit-synthesizes bf16 e^x; sawtooth decorrelates across keys, DC
        cancels in softmax). Halves sit in different PSUM banks so the
        engines read in parallel; PV j0 depends only on the ACT half.
    outT[m, q] = sum_k Vaug[k, m] E[k, q]   PE accumulate over 32 kb.
        Vaug[:, 0:64] = mask*V, Vaug[:, 64] = mask (denominator row).
    Epilogue: PE-transpose outT, divide by denom row, blend with qs
    passthrough for masked queries.

Pipeline: all prep (DMA, K/Q transposes, Vaug) is interleaved into the
main granule loop; emission order per granule is exp(g), QK(g+1), PV(g-1)
(PV delayed one granule) so the in-order PE never stalls the exp engines
behind a PV that waits on exp. A micro-prep fast path (first 2 kb + first
q chunk via f32 HWDGE staging) shortens the ramp; the qc0 epilogue is
spread over 4 granules to avoid a DVE burst. GPSIMD handles the
SBUF-only elementwise prep and the bf16 cast-loads of K/Q.
"""

import numpy as np

import concourse.bacc as bacc
import concourse.bass as bass
import concourse.tile as tile
import concourse.mybir as mybir
from concourse.bass_utils import run_bass_kernel_spmd
from concourse.masks import make_identity

B, S, D = 4, 4096, 64
NCORES = 8
QSH = (B * S) // NCORES          # 2048 queries per core
NKB = S // 128                   # 32 key blocks
NQT = QSH // 128                 # 16 query tiles
QCH = 1024                       # query-chunk width (free dim of scoresT)
NQC = QSH // QCH                 # 2 query chunks
NGR = NQC * NKB                  # 64 granules

F32 = mybir.dt.float32
I16 = mybir.dt.int16
BF16 = mybir.dt.bfloat16
Exp = mybir.ActivationFunctionType.Exp
MUL = mybir.AluOpType.mult
ADD = mybir.AluOpType.add

# Schraudolph constants: int16 bits = round(sc*SCH_A + SCH_B) ~ bf16(e^(sc/8))
LOG2E = 1.4426950408889634
SCH_A = 0.125 * LOG2E * 128.0
SCH_B = 127.0 * 128.0 - 5.76 - 0.5   # sawtooth centering + HW rounding bias (tuned)

# granule -> exp engine: DVE every 2nd granule
DVE_MOD, DVE_PHASE = 2, 1


def _emit(tc, nc, q_d, k_d, v_d, mk_d, mq_d, o_d, dve_mod=DVE_MOD,
          dve_phase=DVE_PHASE, pack=True, use_gpsimd=True, mask_in_exp=True,
          bf16_kq=True, dup=(), half=(), sch_b_off=0.0, pv_delay=True,
          epi_spread=True, dve_res=(1, 5), split_exp=640):
    ew = nc.gpsimd if use_gpsimd else nc.vector
    KQDT = BF16 if bf16_kq else F32
    consts = tc.alloc_tile_pool(name="consts", bufs=1)
    sb = tc.alloc_tile_pool(name="sb", bufs=1)
    expp = tc.alloc_tile_pool(name="expp", bufs=6)
    finp = tc.alloc_tile_pool(name="finp", bufs=4)
    ps = tc.alloc_tile_pool(name="ps", bufs=3, space="PSUM")     # tag "sc"
    ps_o = tc.alloc_tile_pool(name="ps_o", bufs=1, space="PSUM")  # tag "ot"
    pools = [consts, sb, expp, finp, ps, ps_o]

    identity = consts.tile([128, 128], F32, name="identity")
    make_identity(nc, identity)
    # warm the ACT exp table before the pipeline needs it
    actwarm = consts.tile([1, 1], F32, name="actwarm")
    nc.scalar.activation(out=actwarm, in_=identity[0:1, 0:1], func=Exp)

    q3 = sb.tile([128, NQT, D], KQDT, name="q3")     # q natural, qt = rows qt*128+p
    k3 = sb.tile([128, NKB, D], KQDT, name="k3")
    if bf16_kq:
        identb = consts.tile([128, 128], KQDT, name="identb")
        nc.vector.tensor_copy(identb, identity)
    else:
        identb = identity
    v3f = sb.tile([128, NKB, D], F32, name="v3f")    # fp32 staging for V
    v3 = sb.tile([128, NKB, D + 1], BF16, name="v3")  # V augmented with mask col
    mk = sb.tile([128, NKB], F32, name="mk_sb")
    mq = sb.tile([128, NQT], F32, name="mq_sb")
    s1 = sb.tile([128, NQT], F32, name="s1_sb")      # 0.125*(1-mq)
    lm = sb.tile([128, NKB], F32, name="lm_sb")      # 0 keep / -1e30 masked
    bv = sb.tile([128, NKB], F32, name="bv_sb")      # SCH_B keep / 0 masked
    qb3 = sb.tile([128, NQT, D], F32, name="qb3")    # qs*(1-mq) passthrough term
    k3f = sb.tile([128, 2, D], F32, name="k3f")      # ramp: first 2 kb via HWDGE
    q3f = sb.tile([128, 8, D], F32, name="q3f")      # ramp: first q chunk via HWDGE
    qT2 = sb.tile([128, QSH], BF16, name="qT2")      # rows 64-127 duplicate 0-63
    kT2 = sb.tile([128, S], BF16, name="kT2")
    oT_sb = sb.tile([D + 1, QCH], F32, name="oT_sb")

    # ---- input DMAs: few, big, first-needed first ----
    qap = q_d.ap().rearrange("(p n) d -> p n d", p=128)
    kap = k_d.ap().rearrange("(p n) d -> p n d", p=128)
    vap = v_d.ap().rearrange("(p n) d -> p n d", p=128)
    nc.sync.dma_start(out=k3f, in_=kap[:, 0:2, :])
    nc.sync.dma_start(out=q3f, in_=qap[:, 0:8, :])
    kqdma = nc.gpsimd.dma_start if bf16_kq else nc.sync.dma_start
    kqdma(out=k3[:, 0:8, :], in_=kap[:, 0:8, :])
    kqdma(out=q3[:, 0:8, :], in_=qap[:, 0:8, :])
    nc.sync.dma_start(out=mk, in_=mk_d.ap().rearrange("(p n) -> p n", p=128))
    nc.sync.dma_start(out=mq, in_=mq_d.ap().rearrange("(p n) -> p n", p=128))
    nc.sync.dma_start(out=v3f[:, 0:16, :], in_=vap[:, 0:16, :])
    kqdma(out=k3[:, 8:32, :], in_=kap[:, 8:32, :])
    kqdma(out=q3[:, 8:16, :], in_=qap[:, 8:16, :])
    nc.sync.dma_start(out=v3f[:, 16:32, :], in_=vap[:, 16:32, :])

    # s1 = 0.125 * (1 - mq) = mq * (-0.125) + 0.125   (gpsimd: SBUF-only)
    ew.tensor_scalar(s1, mq, -0.125, 0.125, MUL, ADD)
    if mask_in_exp:
        # lm = (mk-1)*1e30 -> {0, -1e30}; bv = mk*SCH_B -> {SCH_B, 0}
        ew.tensor_scalar(lm, mk, 1e30, -1e30, MUL, ADD)
        # masked: bits = A*sc + 4096 stays positive-small -> bf16 ~2^-96
        ew.tensor_scalar(bv, mk, SCH_B + sch_b_off - 4096.0, 4096.0, MUL, ADD)

    def v3_prep(h):
        if mask_in_exp:
            # mask rides in exp: Vaug = plain V (bf16) + ones column
            ew.memset(v3[:, 8 * h:8 * h + 8, D:D + 1], 1.0)
            ew.tensor_copy(v3[:, 8 * h:8 * h + 8, 0:D],
                           v3f[:, 8 * h:8 * h + 8, :])
            return
        # Vaug for kb 8h..8h+7: col D = mask, cols 0:D = mask * V
        ew.tensor_copy(v3[:, 8 * h:8 * h + 8, D:D + 1],
                              mk[:, 8 * h:8 * h + 8].rearrange(
                                  "p (n o) -> p n o", o=1))
        for j in range(8):
            kb = 8 * h + j
            ew.tensor_scalar_mul(v3[:, kb, 0:D], v3f[:, kb, :],
                                        mk[:, kb:kb + 1])

    def qb3_prep(quarter):
        for j in range(4):
            qt = 4 * quarter + j
            ew.tensor_scalar_mul(qb3[:, qt, :], q3[:, qt, :],
                                        s1[:, qt:qt + 1])

    def micro_prep():
        # ramp fast path: kb0-1 + full first q chunk from f32 staging.
        # Transposes must write PSUM partition 0 (walrus rule); the hi
        # halves come from cross-partition DVE copies (verified on HW).
        tpq = ps.tile([128, 1024], F32, name="tpq_micro", tag="sc")
        tpk = ps.tile([128, 256], F32, name="tpk_micro", tag="sc")
        for j in range(8):
            nc.tensor.transpose(tpq[0:64, 128 * j:128 * (j + 1)],
                                q3f[:, j, :], identity)
        for j in range(2):
            nc.tensor.transpose(tpk[0:64, 128 * j:128 * (j + 1)],
                                k3f[:, j, :], identity)
        nc.vector.tensor_copy(qT2[0:64, 0:512], tpq[0:64, 0:512])
        nc.vector.tensor_copy(kT2[0:64, 0:256], tpk[0:64, :])
        if pack:
            nc.vector.tensor_copy(qT2[64:128, 512:1024], tpq[0:64, 512:1024])
            nc.vector.tensor_copy(kT2[64:128, 0:256], tpk[0:64, :])
        else:
            nc.vector.tensor_copy(qT2[0:64, 512:1024], tpq[0:64, 512:1024])

    def prep_kt(gi, kb_lo=0):
        # transpose kb 8gi+kb_lo..8gi+7 into kT2[0:64]; duplicate into
        # [64:128] (group 0: cross-partition DVE copy, no DMA latency; later
        # groups: SBUF->SBUF DMA off the critical path)
        tp = ps.tile([128, 1024], KQDT, name=f"prepk{gi}", tag="sc")
        for j in range(kb_lo, 8):
            kb = 8 * gi + j
            nc.tensor.transpose(tp[0:64, 128 * j:128 * (j + 1)], k3[:, kb, :],
                                identb)
        lo = kT2[0:64, 1024 * gi + 128 * kb_lo:1024 * (gi + 1)]
        nc.vector.tensor_copy(lo, tp[0:64, 128 * kb_lo:])
        if pack:
            hi = kT2[64:128, 1024 * gi + 128 * kb_lo:1024 * (gi + 1)]
            if gi == 0:
                nc.vector.tensor_copy(hi, tp[0:64, 128 * kb_lo:])
            else:
                nc.sync.dma_start(out=hi, in_=lo)

    def prep_qt(gi):
        # QK tile-A only reads qT2[0:64, chunk+0:512]; tile-B only reads
        # qT2[64:128, chunk+512:1024]. Transpose each qt straight into the
        # half that will read it -> half the copy work, no duplication.
        tp = ps.tile([128, 1024], KQDT, name=f"prepq{gi}", tag="sc")
        base = 1024 * gi
        for j in range(8):
            qt = 8 * gi + j
            nc.tensor.transpose(tp[0:64, 128 * j:128 * (j + 1)],
                                q3[:, qt, :], identb)
        nc.vector.tensor_copy(qT2[0:64, base:base + 512], tp[0:64, 0:512])
        if pack:
            nc.vector.tensor_copy(qT2[64:128, base + 512:base + 1024],
                                  tp[0:64, 512:1024])
        else:
            nc.vector.tensor_copy(qT2[0:64, base + 512:base + 1024],
                                  tp[0:64, 512:1024])

    # minimal pre-loop prep: micro (kb0-1 + q chunk 0), then bulk kb2-7
    micro_prep()
    prep_kt(0, kb_lo=2)
    v3_prep(0)

    oT = [None, None]

    def emit_qk(g):
        qc, kb = g // NKB, g % NKB
        sc = ps.tile([128, QCH], F32, name=f"sc{g}", tag="sc")
        nc.tensor.matmul(
            sc[:, 0:512],
            lhsT=kT2[0:64, 128 * kb:128 * (kb + 1)],
            rhs=qT2[0:64, QCH * qc:QCH * qc + 512],
            start=True, stop=True, tile_position=(0, 0))
        hb = 64 if pack else 0
        if "qk" not in half:
            nc.tensor.matmul(
                sc[:, 512:1024],
                lhsT=kT2[hb:hb + 64, 128 * kb:128 * (kb + 1)],
                rhs=qT2[hb:hb + 64, QCH * qc + 512:QCH * qc + 1024],
                start=True, stop=True, tile_position=(hb, 0))
        if "qk" in dup:
            nc.tensor.matmul(
                sc[:, 0:512], lhsT=kT2[0:64, 128 * kb:128 * (kb + 1)],
                rhs=qT2[0:64, QCH * qc:QCH * qc + 512],
                start=True, stop=True, tile_position=(0, 0))
            nc.tensor.matmul(
                sc[:, 512:1024], lhsT=kT2[hb:hb + 64, 128 * kb:128 * (kb + 1)],
                rhs=qT2[hb:hb + 64, QCH * qc + 512:QCH * qc + 1024],
                start=True, stop=True, tile_position=(hb, 0))
        return sc

    if dve_res is not None:
        dve_set = set(g for g in range(NGR) if g % 8 in dve_res)
    else:
        dve_set = set(g for g in range(NGR)
                      if dve_mod and g % dve_mod == dve_phase)
    # keep DVE clear while it chews the qc0 epilogue burst
    if epi_spread:
        for g_excl, g_repl in ((35, 33), (38, 39)):
            if g_excl in dve_set:
                dve_set.discard(g_excl)
                dve_set.add(g_repl)

    def emit_exp(g, sc):
        kb = g % NKB
        if split_exp:
            # both engines on every granule: ACT cols [0:split], DVE the rest.
            # Different PSUM banks -> parallel reads; PV j0 only needs ACT half.
            exi = expp.tile([128, QCH], I16, name=f"ex{g}", tag="ex")
            bias = lm[:, kb:kb + 1] if mask_in_exp else 0.0
            nc.scalar.activation(out=exi[:, 0:split_exp].bitcast(BF16),
                                 in_=sc[:, 0:split_exp], func=Exp,
                                 scale=0.125, bias=bias)
            schb = bv[:, kb:kb + 1] if mask_in_exp else SCH_B + sch_b_off
            nc.vector.tensor_scalar(exi[:, split_exp:QCH],
                                    sc[:, split_exp:QCH], SCH_A, schb,
                                    MUL, ADD)
            return exi.bitcast(BF16)
        if g in dve_set:
            exi = expp.tile([128, QCH], I16, name=f"ex{g}", tag="ex")
            schb = bv[:, kb:kb + 1] if mask_in_exp else SCH_B + sch_b_off
            nc.vector.tensor_scalar(exi[:, 0:512] if "exp" in half else exi,
                                    sc[:, 0:512] if "exp" in half else sc,
                                    SCH_A, schb, MUL, ADD)
            if "exp" in dup:
                nc.vector.tensor_scalar(exi, sc, SCH_A, schb, MUL, ADD)
            return exi.bitcast(BF16)
        ex = expp.tile([128, QCH], BF16, name=f"ex{g}", tag="ex")
        bias = lm[:, kb:kb + 1] if mask_in_exp else 0.0
        nc.scalar.activation(out=ex[:, 0:512] if "exp" in half else ex,
                             in_=sc[:, 0:512] if "exp" in half else sc,
                             func=Exp, scale=0.125, bias=bias)
        if "exp" in dup:
            nc.scalar.activation(out=ex, in_=sc, func=Exp, scale=0.125, bias=bias)
        return ex

    def emit_pv(g, ex):
        qc, kb = g // NKB, g % NKB
        if kb == 0:
            oT[qc] = ps_o.tile([D + 1, QCH], F32, name=f"oT{qc}", tag="ot")
        for _r in range(2 if "pv" in dup else 1):
            for j in range(1 if "pv" in half else 2):
                nc.tensor.matmul(
                    oT[qc][:, 512 * j:512 * (j + 1)],
                    lhsT=v3[:, kb, :],
                    rhs=ex[:, 512 * j:512 * (j + 1)],
                    start=(kb == 0 and _r == 0), stop=(kb == NKB - 1))

    epist = {}

    def emit_epilogue(qc, chunk=None, nchunks=1):
        # oT -> SBUF, per-qt transpose back, divide by denom, blend, DMA out
        nqt = QCH // 128
        if chunk is None or chunk == 0:
            nc.vector.tensor_copy(oT_sb, oT[qc])
            epist[qc] = (
                ps.tile([128, nqt, 128], F32, name=f"epi{qc}", tag="sc"),
                finp.tile([128, nqt, D], F32, name=f"fin3_{qc}", tag="fin3",
                          bufs=2))
        epi, fin3 = epist[qc]
        per = nqt // nchunks
        ts = range(nqt) if chunk is None else range(chunk * per,
                                                   (chunk + 1) * per)
        for t in ts:
            qt = qc * nqt + t
            nc.tensor.transpose(epi[:, t, 0:D + 1],
                                oT_sb[:, 128 * t:128 * (t + 1)],
                                identity[0:D + 1, 0:D + 1])
            rec = finp.tile([128, 1], F32, name=f"rec{qt}", tag="rec")
            nc.vector.reciprocal(rec, epi[:, t, D:D + 1])
            recm = finp.tile([128, 1], F32, name=f"recm{qt}", tag="recm")
            nc.vector.tensor_scalar_mul(recm, rec, mq[:, qt:qt + 1])
            # fin = (pv * recm) + qb
            nc.vector.scalar_tensor_tensor(fin3[:, t, :], epi[:, t, 0:D], recm,
                                           qb3[:, qt, :], MUL, ADD)
        if chunk is None or chunk == nchunks - 1:
            oap = o_d.ap().rearrange("(p n) d -> p n d", p=128)
            nc.sync.dma_start(
                out=oap[:, qc * nqt:(qc + 1) * nqt, :], in_=fin3)

    # interleaved prep: granule index -> thunk
    inter = {
        1: lambda: prep_kt(1),
        3: lambda: v3_prep(1),
        5: lambda: prep_kt(2),
        7: lambda: v3_prep(2),
        9: lambda: prep_kt(3),
        11: lambda: v3_prep(3),
        13: lambda: prep_qt(1),
        24: lambda: qb3_prep(0),
        26: lambda: qb3_prep(1),
        28: lambda: qb3_prep(2),
        30: lambda: qb3_prep(3),
        34: lambda: emit_epilogue(0, 0, 4),
        35: lambda: emit_epilogue(0, 1, 4),
        36: lambda: emit_epilogue(0, 2, 4),
        37: lambda: emit_epilogue(0, 3, 4),
    } if epi_spread else {
        1: lambda: prep_kt(1),
        3: lambda: v3_prep(1),
        5: lambda: prep_kt(2),
        7: lambda: v3_prep(2),
        9: lambda: prep_kt(3),
        11: lambda: v3_prep(3),
        13: lambda: prep_qt(1),
        24: lambda: qb3_prep(0),
        26: lambda: qb3_prep(1),
        28: lambda: qb3_prep(2),
        30: lambda: qb3_prep(3),
        34: lambda: emit_epilogue(0),
    }

    # ---- main loop: exp(g) / QK(g+1) / PV(g-1) ----
    # PV delayed one granule so the in-order PE runs QK(g+1) before
    # PV(g-1); exp(g+1) then never waits behind PV in the PE stream.
    sc_cur = emit_qk(0)
    ex_prev = None
    for g in range(NGR):
        ex = emit_exp(g, sc_cur)
        if g + 1 < NGR:
            sc_cur = emit_qk(g + 1)
        if pv_delay:
            if ex_prev is not None:
                emit_pv(g - 1, ex_prev)
            ex_prev = ex
        else:
            emit_pv(g, ex)
        thunk = inter.get(g)
        if thunk is not None:
            thunk()
    if pv_delay:
        emit_pv(NGR - 1, ex_prev)
    emit_epilogue(1)

    for p in reversed(pools):
        p.release()


_PROGS = {}


def _build(repeat=1, loop=None, **emit_kwargs):
    key = (repeat, loop, tuple(sorted(emit_kwargs.items())))
    if key in _PROGS:
        return _PROGS[key]
    nc = bacc.Bacc("TRN2", target_bir_lowering=False, debug=False)
    q_d = nc.dram_tensor("q_in", [QSH, D], F32, kind="ExternalInput")
    k_d = nc.dram_tensor("k_in", [S, D], F32, kind="ExternalInput")
    v_d = nc.dram_tensor("v_in", [S, D], F32, kind="ExternalInput")
    mk_d = nc.dram_tensor("mk_in", [S], F32, kind="ExternalInput")
    mq_d = nc.dram_tensor("mq_in", [QSH], F32, kind="ExternalInput")
    o_d = nc.dram_tensor("o_out", [QSH, D], F32, kind="ExternalOutput")
    with tile.TileContext(nc) as tc:
        if loop is not None:
            with tc.For_i(0, loop, 1):
                for _ in range(repeat):
                    _emit(tc, nc, q_d, k_d, v_d, mk_d, mq_d, o_d, **emit_kwargs)
        else:
            for _ in range(repeat):
                _emit(tc, nc, q_d, k_d, v_d, mk_d, mq_d, o_d, **emit_kwargs)
    nc.compile()
    _PROGS[key] = nc
    return nc


def make_in_maps(q, k, v, mask):
    q = np.ascontiguousarray(np.asarray(q, dtype=np.float32))
    k = np.ascontiguousarray(np.asarray(k, dtype=np.float32))
    v = np.ascontiguousarray(np.asarray(v, dtype=np.float32))
    mask = np.ascontiguousarray(np.asarray(mask, dtype=np.float32))
    in_maps = []
    for c in range(NCORES):
        b, h = c // 2, c % 2
        sl = slice(h * QSH, (h + 1) * QSH)
        in_maps.append({
            "q_in": np.ascontiguousarray(q[b, sl, :]),
            "k_in": np.ascontiguousarray(k[b]),
            "v_in": np.ascontiguousarray(v[b]),
            "mk_in": np.ascontiguousarray(mask[b]),
            "mq_in": np.ascontiguousarray(mask[b, sl]),
        })
    return in_maps


def gather(results):
    out = np.empty((B, S, D), np.float32)
    for c in range(NCORES):
        b, h = c // 2, c % 2
        out[b, h * QSH:(h + 1) * QSH, :] = results[c]["o_out"]
    return out


def kernel(q, k, v, mask, _spmd_kwargs=None):
    nc = _build()
    in_maps = make_in_maps(q, k, v, mask)
    res = run_bass_kernel_spmd(nc, in_maps, core_ids=list(range(NCORES)),
                               **(_spmd_kwargs or {}))
    out = gather(res.results)
    if _spmd_kwargs:
        kernel._last_results = res
    return out
